# revision 10
# baseline (speedup 1.0000x reference)
"""DiGCN inception-block (3 layers, 2 adjacencies) on 8 TRN2 NeuronCores — v4.

Architecture ("q-phased with DRAM-staged partials"):
  The SWDGE gather path is descriptor-rate limited (~110GB/s for 256B
  rows, measured), so the design keeps gather descriptors flowing
  CONTINUOUSLY across layer boundaries instead of idling during layer 1
  (host-streamed) and at AllGather boundaries.

  - The node table is split into 4 quarter tensors, each written by one
    AllGather chunk, and chunk boundaries coincide with the gather
    q-ranges (int16 srel fits: max range 26624 rows).  A gather for
    range q depends only on quarter tensor q.
  - Layers 2/3 emit gathers q-major: all range-0 gathers for the whole
    layer, then range-1, etc.  Range q of layer k+1 becomes available
    roughly when quarter q of layer k completes, so the gather engine
    always has runnable work.
  - A block's aggregation needs all 4 ranges, so per-(block, range)
    partial aggregates st_q are computed as each range arrives and
    staged to DRAM (the pipeline buffer; SBUF can't hold the prefetch
    window).  The q3 partial skips the round-trip: the dense phase is
    inlined into the q3 run and consumes it from SBUF, re-reading only
    the q0-2 partials.
  - Gather/idx/staging tiles alternate between two tag sets by layer
    parity so pool rotation never chains layer k+1's first gather to
    layer k's last consumer.
  - One-hot scatter matrices (a pure layout transform of edge_attr)
    are host-built and streamed from DRAM; DVE does copies/adds only.
  - Layer 1 has no gathers (host-streamed gstr) and no DRAM staging:
    per-range partials accumulate into an SBUF tile per group (DVE
    adds), then dense runs directly.
"""

import sys

sys.path.insert(0, "/opt/trn_rl_repo")

import numpy as np
import ml_dtypes

from concourse import bass, mybir, bacc
import concourse.tile as tile
from concourse.bass_utils import run_bass_kernel_spmd

BF16 = ml_dtypes.bfloat16

NCORES = 8
F = 128
N = 100000
NPAD = 100352   # 8 * 12544
NL = NPAD // NCORES          # 12544 rows per core
B = NL // 128                # 98 blocks per core
R = 4                        # table quarters / gather ranges
SBG = 4                      # blocks per group (quarter bounds are multiples)
QBLK = [0, 24, 48, 72, 98]   # block boundaries of quarters (per core)
SCRATCH = 49152
MAXCH = 8                    # chunks per gather call (1024-idx ucode cap)


def _groups():
    out = []
    b0 = 0
    while b0 < B:
        out.append((b0, min(SBG, B - b0)))
        b0 += SBG
    return out


GROUPS = _groups()           # 25 groups: 24x4 + 1x2
GQUART = [next(c for c in range(4) if b0 < QBLK[c + 1]) for b0, _ in GROUPS]
QROWS_SHARD = [(QBLK[c + 1] - QBLK[c]) * 128 for c in range(4)]
QROWS = [r * NCORES for r in QROWS_SHARD]
QBASE = np.concatenate([[0], np.cumsum(QROWS)]).astype(np.int64)


def _pos_map():
    """Quarter-major permuted node position: table[POS[v]] = x[v]."""
    POS = np.empty(NPAD, np.int64)
    base = 0
    for c in range(4):
        r0, r1 = QBLK[c] * 128, QBLK[c + 1] * 128
        ch = r1 - r0
        for r in range(NCORES):
            v = r * NL + np.arange(r0, r1)
            POS[v] = base + r * ch + np.arange(ch)
        base += NCORES * ch
    return POS


def _prep_adjacency(src, dst, attr, POS):
    """Bucket each core's edges by (dest block, source quarter)."""
    per_core = []
    core = dst // NL
    pos = POS[src]
    q_all = np.searchsorted(QBASE, pos, side="right") - 1
    srel_all = pos - QBASE[q_all]
    for r in range(NCORES):
        m = core == r
        d = (dst[m] - r * NL).astype(np.int64)
        b = d >> 7
        drel = (d & 127).astype(np.int64)
        q = q_all[m]
        srel = srel_all[m]
        a = attr[m].astype(np.float32)
        key = b * R + q
        order = np.argsort(key, kind="stable")
        key_s = key[order]
        counts = np.bincount(key_s, minlength=B * R)
        starts = np.concatenate([[0], np.cumsum(counts)[:-1]])
        pos_in = np.arange(len(key_s)) - starts[key_s]
        per_core.append((key_s, pos_in, srel[order], drel[order], a[order],
                         counts))
    return per_core


class Layout:
    """Build-time chunk offsets shared by host packing and kernel build.

    kap[a][b,q]: chunks for bucket (b,q) = ceil(max-over-cores count/128),
    shared across cores so the SPMD program is uniform.
    Chunk order: [q][group][block][s]."""

    def __init__(self, adjs):
        self.kap = []
        for pc in adjs:
            mx = np.max(np.stack([c[5] for c in pc]), axis=0).reshape(B, R)
            self.kap.append(np.ceil(mx / 128).astype(np.int64))
        nG = len(GROUPS)
        self.toff = [[[0] * nG for _ in range(R)] for _ in range(2)]
        self.ncg = [[[0] * nG for _ in range(R)] for _ in range(2)]
        self.boff = [[[None] * nG for _ in range(R)] for _ in range(2)]
        self.tot = [0, 0]
        for a in range(2):
            off = 0
            for q in range(R):
                for g, (b0, nb) in enumerate(GROUPS):
                    bo = []
                    n = 0
                    for b in range(b0, b0 + nb):
                        bo.append(n)
                        n += int(self.kap[a][b, q])
                    self.boff[a][q][g] = bo
                    self.toff[a][q][g] = off
                    self.ncg[a][q][g] = n
                    off += n
            self.tot[a] = off
        self.maxncg = max(self.ncg[a][q][g] for a in range(2)
                          for q in range(R) for g in range(len(GROUPS)))


def _finalize(per_core, lay, a):
    """Pack per-core idx tokens, one-hot grids, gather-stream positions
    in the shared [q][group][block][chunk] order."""
    kap = lay.kap[a]
    # chunk base offset per (b, q) in final order
    base = np.zeros((B, R), np.int64)
    for q in range(R):
        for g, (b0, nb) in enumerate(GROUPS):
            for bi, b in enumerate(range(b0, b0 + nb)):
                base[b, q] = lay.toff[a][q][g] + lay.boff[a][q][g][bi]
    ncap = lay.tot[a]
    idx_arrs, oh_arrs, gpos_arrs = [], [], []
    qof_chunk = np.zeros(ncap, np.int64)       # QBASE of each chunk's range
    for q in range(R):
        for g in range(len(GROUPS)):
            t0 = lay.toff[a][q][g]
            qof_chunk[t0:t0 + lay.ncg[a][q][g]] = QBASE[q]
    for key_s, pos_in, srel, drel, attr, counts in per_core:
        b_ = key_s // R
        q_ = key_s % R
        chunk = base[b_, q_] + pos_in // 128
        p_ = pos_in % 128
        grid_src = np.zeros((ncap, 128), np.int64)
        grid_oh = np.zeros((ncap, 128, 128), BF16)
        grid_src[chunk, p_] = srel
        grid_oh[chunk, p_, drel] = attr.astype(BF16)
        tokens = grid_src.reshape(-1)
        gpos_arrs.append(tokens + np.repeat(qof_chunk, 128))
        wrapped = np.tile(tokens.astype(np.int16).reshape(-1, 16).T, (8, 1))
        idx_arrs.append(np.ascontiguousarray(wrapped))
        oh_arrs.append(np.ascontiguousarray(
            grid_oh.transpose(1, 0, 2).reshape(128, ncap * 128)))
    return idx_arrs, oh_arrs, gpos_arrs


def _build_kernel(lay):
    dt = mybir.dt
    nc = bacc.Bacc("TRN2", target_bir_lowering=False, debug=False,
                   num_devices=NCORES, num_swdge_queues=4,
                   dynamic_dma_scratch_size=SCRATCH)

    xT0_in = nc.declare_dram_parameter("input0", [128, NL], dt.bfloat16,
                                       isOutput=False)
    idx_in = [nc.declare_dram_parameter(f"input{1 + i}",
                                        [128, lay.tot[i] * 8], dt.int16,
                                        isOutput=False) for i in range(2)]
    oh_in = [nc.declare_dram_parameter(f"input{3 + i}",
                                       [128, lay.tot[i] * 128], dt.bfloat16,
                                       isOutput=False) for i in range(2)]
    w_in = nc.declare_dram_parameter("input5", [9 * 128, F], dt.bfloat16,
                                     isOutput=False)
    bias_in = nc.declare_dram_parameter("input6", [128, 3 * F], dt.float32,
                                        isOutput=False)
    ident_in = nc.declare_dram_parameter("input7", [128, 128], dt.bfloat16,
                                         isOutput=False)
    gstr_in = [nc.declare_dram_parameter(f"input{8 + i}",
                                         [128, lay.tot[i] * F], dt.bfloat16,
                                         isOutput=False) for i in range(2)]
    out_p = nc.declare_dram_parameter("output0", [NL, F], dt.float32,
                                      isOutput=True)

    qt = [[nc.dram_tensor(f"qt{k}_{c}", [QROWS[c], F], dt.bfloat16,
                          addr_space="Shared") for c in range(4)]
          for k in range(2)]
    shard = [nc.dram_tensor(f"shard{k}", [NL, F], dt.bfloat16)
             for k in range(2)]
    # staged partials for q0-2: [parity][a][q] -> [128, NL] (col b*128+d)
    stq = [[[nc.dram_tensor(f"stq{p}_{a}_{q}", [128, NL], dt.bfloat16)
             for q in range(3)] for a in range(2)] for p in range(2)]

    with tile.TileContext(nc) as tc:
        with (
            tc.tile_pool(name="persist", bufs=1) as pp,
            tc.tile_pool(name="idxp", bufs=2) as idxp,
            tc.tile_pool(name="gts0", bufs=2) as gtp0,
            tc.tile_pool(name="gts1", bufs=2) as gtp1,
            tc.tile_pool(name="ohp", bufs=2) as ohp,
            tc.tile_pool(name="stgp", bufs=2) as stgp,
            tc.tile_pool(name="srp", bufs=2) as srp,
            tc.tile_pool(name="accp", bufs=2) as accp,
            tc.tile_pool(name="outp", bufs=4) as outp,
            tc.tile_pool(name="psA", bufs=6, space="PSUM") as psA,
            tc.tile_pool(name="psB", bufs=2, space="PSUM") as psB,
        ):
            gtp = [gtp0, gtp1]
            ident_t = pp.tile([128, 128], dt.bfloat16, tag="ident")
            nc.sync.dma_start(ident_t[:], ident_in[:])
            w_t = pp.tile([128, 9, 128], dt.bfloat16, tag="w")
            nc.sync.dma_start(w_t[:], w_in[:].rearrange("(w i) o -> i w o",
                                                        i=128))
            bias_t = pp.tile([128, 3 * F], dt.float32, tag="bias")
            nc.sync.dma_start(bias_t[:], bias_in[:])
            xT = pp.tile([128, NL], dt.bfloat16, tag="xT")
            nc.sync.dma_start(xT[:], xT0_in[:])

            def load_oh(a, q, g):
                ncg = lay.ncg[a][q][g]
                t0 = lay.toff[a][q][g]
                oh = ohp.tile([128, lay.maxncg * 128], dt.bfloat16,
                              tag=f"oh{a}", name=f"oh{a}")
                nc.scalar.dma_start(oh[:, :ncg * 128],
                                    oh_in[a][:, t0 * 128:(t0 + ncg) * 128])
                return oh

            def load_gts(a, q, g, par, k):
                """k==0: sequential stream; k>0: SWDGE gather from qt."""
                ncg = lay.ncg[a][q][g]
                t0 = lay.toff[a][q][g]
                gt = gtp[par].tile([128, lay.maxncg, F], dt.bfloat16,
                                   tag=f"g{a}", name=f"g{a}")
                if ncg == 0:
                    return gt
                if k == 0:
                    nc.sync.dma_start(gt[:, :ncg, :],
                                      gstr_in[a][:, t0 * F:(t0 + ncg) * F])
                    return gt
                it = idxp.tile([128, lay.maxncg * 8], dt.int16,
                               tag=f"idx{a}{par}", name=f"idx{a}{par}")
                nc.sync.dma_start(it[:, :ncg * 8],
                                  idx_in[a][:, t0 * 8:(t0 + ncg) * 8])
                c0 = 0
                while c0 < ncg:
                    ncall = min(MAXCH, ncg - c0)
                    nc.gpsimd.dma_gather(
                        out_ap=gt[:, c0:c0 + ncall, :],
                        in_ap=qt[k - 1][q][:],
                        idxs_ap=it[:, c0 * 8:(c0 + ncall) * 8],
                        num_idxs=ncall * 128,
                        num_idxs_reg=ncall * 128,
                        elem_size=F,
                        queue_num=q,
                    )
                    c0 += ncall
                return gt

            def scatter_chains(a, q, g, gt, oh, emit):
                """Per block of group g: psum chain over its chunks, then
                emit(bl, psum_tile). Skips blocks with zero chunks."""
                b0, nb = GROUPS[g]
                bo = lay.boff[a][q][g]
                for bl in range(nb):
                    nchunk = int(lay.kap[a][b0 + bl, q])
                    if nchunk == 0:
                        emit(bl, None)
                        continue
                    s0 = bo[bl]
                    p = psA.tile([128, 128], dt.float32, tag="psA", name="psA")
                    for s in range(nchunk):
                        nc.tensor.matmul(
                            p[:], gt[:, s0 + s, :],
                            oh[:, (s0 + s) * 128:(s0 + s + 1) * 128],
                            start=(s == 0), stop=(s == nchunk - 1))
                    emit(bl, p)

            def dense_block(k, b, mms):
                """po = sum(mms as lhsT @ W[c-branch]) + x @ W0 (+bias)."""
                sl = slice(b * 128, (b + 1) * 128)
                po = psB.tile([128, F], dt.float32, tag="psB", name="psB")
                for i, (lh, wi) in enumerate(mms):
                    nc.tensor.matmul(po[:], lh, w_t[:, k * 3 + wi, :],
                                     start=(i == 0), stop=False)
                nc.tensor.matmul(po[:], xT[:, sl], w_t[:, k * 3 + 0, :],
                                 start=(len(mms) == 0), stop=True)
                if k < 2:
                    ob = outp.tile([128, F], dt.bfloat16, tag="ob", name="ob")
                    nc.vector.tensor_tensor(
                        out=ob[:], in0=po[:], in1=bias_t[:, k * F:(k + 1) * F],
                        op=mybir.AluOpType.add)
                    nc.sync.dma_start(shard[k][sl, :], ob[:])
                    pt = psB.tile([128, F], dt.float32, tag="psB", name="psT")
                    nc.tensor.matmul(pt[:], ob[:], ident_t[:],
                                     start=True, stop=True)
                    nc.scalar.copy(xT[:, sl], pt[:])
                else:
                    ob = outp.tile([128, F], dt.float32, tag="obf", name="obf")
                    nc.vector.tensor_tensor(
                        out=ob[:], in0=po[:], in1=bias_t[:, k * F:(k + 1) * F],
                        op=mybir.AluOpType.add)
                    nc.sync.dma_start(out_p[sl, :], ob[:])

            def maybe_ag(k, b_end):
                if k >= 2:
                    return
                for c in range(4):
                    if QBLK[c + 1] == b_end:
                        r0, r1 = QBLK[c] * 128, QBLK[c + 1] * 128
                        nc.gpsimd.collective_compute(
                            "AllGather", mybir.AluOpType.bypass,
                            replica_groups=[list(range(NCORES))],
                            ins=[shard[k][r0:r1, :]],
                            outs=[qt[k][c][:]],
                        )

            # ---------------- Layer 1: streamed, SBUF-accumulated -------
            for g, (b0, nb) in enumerate(GROUPS):
                acc = {a: accp.tile([128, SBG * 128], dt.bfloat16,
                                    tag=f"acc{a}", name=f"acc{a}")
                       for a in range(2)}
                seen = {}
                for q in range(R):
                    for a in range(2):
                        gt = load_gts(a, q, g, 0, 0)
                        oh = load_oh(a, q, g)

                        def emit(bl, p, a=a):
                            if p is None:
                                return
                            dst = acc[a][:, bl * 128:(bl + 1) * 128]
                            if (a, bl) in seen:
                                nc.vector.tensor_tensor(
                                    out=dst, in0=dst, in1=p[:],
                                    op=mybir.AluOpType.add)
                            else:
                                nc.vector.tensor_copy(out=dst, in_=p[:])
                                seen[(a, bl)] = True
                        scatter_chains(a, q, g, gt, oh, emit)
                for bl in range(nb):
                    mms = []
                    for a in range(2):
                        if (a, bl) in seen:
                            mms.append(
                                (acc[a][:, bl * 128:(bl + 1) * 128], a + 1))
                    dense_block(0, b0 + bl, mms)
                maybe_ag(0, b0 + nb)

            # ---------------- Layers 2/3: q-phased, DRAM staging --------
            for k in (1, 2):
                par = k % 2
                q3stg = {}
                for q in range(R):
                    for g, (b0, nb) in enumerate(GROUPS):
                        for a in range(2):
                            gt = load_gts(a, q, g, par, k)
                            oh = load_oh(a, q, g)
                            stg = stgp.tile([128, SBG * 128], dt.bfloat16,
                                            tag=f"stg{a}{par}",
                                            name=f"stg{a}{par}")

                            def emit(bl, p, stg=stg):
                                dst = stg[:, bl * 128:(bl + 1) * 128]
                                if p is None:
                                    nc.vector.memset(dst, 0)
                                else:
                                    nc.vector.tensor_copy(out=dst, in_=p[:])
                            scatter_chains(a, q, g, gt, oh, emit)
                            if q < 3:
                                nc.sync.dma_start(
                                    stq[par][a][q][:,
                                                   b0 * 128:(b0 + nb) * 128],
                                    stg[:, :nb * 128])
                            else:
                                q3stg[(a, g)] = stg
                        if q == 3:
                            # dense phase for this group, inline in q3 run
                            srt = {}
                            for a in range(2):
                                for qq in range(3):
                                    t = srp.tile([128, SBG * 128], dt.bfloat16,
                                                 tag=f"sr{a}{qq}",
                                                 name=f"sr{a}{qq}")
                                    nc.scalar.dma_start(
                                        t[:, :nb * 128],
                                        stq[par][a][qq][:, b0 * 128:
                                                        (b0 + nb) * 128])
                                    srt[(a, qq)] = t
                            for bl in range(nb):
                                mms = []
                                for a in range(2):
                                    for qq in range(3):
                                        mms.append(
                                            (srt[(a, qq)][:, bl * 128:
                                                          (bl + 1) * 128],
                                             a + 1))
                                    mms.append(
                                        (q3stg[(a, g)][:, bl * 128:
                                                       (bl + 1) * 128],
                                         a + 1))
                                dense_block(k, b0 + bl, mms)
                            maybe_ag(k, b0 + nb)

    from concourse.tile_scheduler import PROC_NAME_TO_IDX
    idx2name = {v: k for k, v in PROC_NAME_TO_IDX.items()}
    for fn in nc.m.functions:
        for block in fn.blocks:
            for inst in block.instructions:
                if isinstance(inst, mybir.InstDMAGatherAnt):
                    pname = str(idx2name.get(inst.bass_scheduled_proc, ""))
                    if pname.startswith("DMASW"):
                        inst.queue_num = int(pname[5:]) % 4
    nc.finalize()
    return nc


def _run(x, edge_index, edge_attr, edge_index2, edge_attr2, weights, biases,
         NPAD_, trace=False):
    n = x.shape[0]
    POS = _pos_map()

    adjs = []
    for (src, dst), attr in ((edge_index, edge_attr), (edge_index2,
                                                       edge_attr2)):
        adjs.append(_prep_adjacency(
            np.asarray(src, np.int64), np.asarray(dst, np.int64), attr, POS))
    lay = Layout(adjs)
    data = [_finalize(adjs[a], lay, a) for a in range(2)]

    xpad = np.zeros((NPAD, x.shape[1]), np.float32)
    xpad[:n] = x
    xtab = np.zeros((NPAD, x.shape[1]), np.float32)
    xtab[POS] = xpad
    xtab = xtab.astype(BF16)
    xpad_bf = xpad.astype(BF16)

    wstack = np.concatenate(
        [np.asarray(w, np.float32) for trio in weights for w in trio], axis=0
    ).astype(BF16)
    bstack = np.concatenate(
        [np.tile(np.asarray(b, np.float32)[None, :], (128, 1)) for b in biases],
        axis=1).astype(np.float32)
    ident = np.eye(128, dtype=np.float32).astype(BF16)

    in_maps = []
    for r in range(NCORES):
        xT0 = np.ascontiguousarray(xpad_bf[r * NL:(r + 1) * NL].T)
        gstr = []
        for a in range(2):
            gpos = data[a][2][r]
            rows = xtab[gpos]
            S = lay.tot[a]
            gstr.append(np.ascontiguousarray(
                rows.reshape(S, 128, F).transpose(1, 0, 2).reshape(128,
                                                                   S * F)))
        in_maps.append({
            "input0": xT0,
            "input1": data[0][0][r],
            "input2": data[1][0][r],
            "input3": data[0][1][r],
            "input4": data[1][1][r],
            "input5": wstack,
            "input6": bstack,
            "input7": ident,
            "input8": gstr[0],
            "input9": gstr[1],
        })

    nc = _build_kernel(lay)
    res = run_bass_kernel_spmd(nc, in_maps, list(range(NCORES)), trace=trace)
    out = np.concatenate([res.results[r]["output0"] for r in range(NCORES)],
                         axis=0)
    return out[:n], res


def kernel(**inputs):
    x = np.asarray(inputs["x"], np.float32)
    weights = []
    biases = []
    for blk in ("b1", "b2", "b3"):
        weights.append(
            (
                np.asarray(inputs[f"{blk}_ln_w"], np.float32),
                np.asarray(inputs[f"{blk}_c1_w"], np.float32),
                np.asarray(inputs[f"{blk}_c2_w"], np.float32),
            )
        )
        biases.append(
            np.asarray(inputs[f"{blk}_ln_b"], np.float32)
            + np.asarray(inputs[f"{blk}_c1_b"], np.float32)
            + np.asarray(inputs[f"{blk}_c2_b"], np.float32)
        )
    out, _ = _run(
        x,
        np.asarray(inputs["edge_index"]),
        np.asarray(inputs["edge_attr"], np.float32),
        np.asarray(inputs["edge_index2"]),
        np.asarray(inputs["edge_attr2"], np.float32),
        weights,
        biases,
        NPAD,
    )
    return out


# revision 11
# speedup vs baseline: 1.7061x; 1.7061x over previous
"""DiGCN inception-block (3 layers, 2 adjacencies) on 8 TRN2 NeuronCores — v5.

v3 (host-streamed one-hot) + layer-1 load split: layer 1 is HWDGE-
bandwidth-bound (~300GB/s streaming gstr+oh) while the SWDGE gather path
(~110GB/s, descriptor-rate limited) idles.  Offload 2 of the 8 (adj, q)
gather-stream slices of layer 1 to SWDGE gathers from the (input) node
table so both paths run concurrently.

  - One-hot scatter matrices are a pure layout transform of edge_attr:
    host-built, streamed from DRAM per (adjacency, block) on the
    Activation-engine HWDGE queue.  DVE only does bias adds.
  - Per-layer transpose DMA replaced by inline PE transposes into the
    persistent xT tile.
"""

import sys

sys.path.insert(0, "/opt/trn_rl_repo")

import numpy as np
import ml_dtypes

from concourse import bass, mybir, bacc
import concourse.tile as tile
from concourse.bass_utils import run_bass_kernel_spmd

BF16 = ml_dtypes.bfloat16

NCORES = 8
F = 128
N = 100000
NPAD = 100352  # 8 * 12544
R = 4
SB = 4           # blocks per superblock
SCRATCH = 49152  # swdge descriptor ring: 3072 descs/queue
L1_GATHER = ((0, 0), (0, 1))   # (a, q) slices of layer 1 fed by SWDGE


def _sb_ranges(B):
    out = []
    b0 = 0
    while b0 < B:
        nb = min(SB, B - b0)
        out.append((b0, nb))
        b0 += nb
    return out


def _chunks(B):
    """AllGather chunking: returns (sbr, bounds, rows)."""
    sbr = _sb_ranges(B)
    n_sb = len(sbr)
    bounds = sorted({max(1, n_sb // 4), max(1, n_sb // 2),
                     max(1, (3 * n_sb) // 4), n_sb})
    rows = []
    prev = 0
    for b in bounds:
        r1 = sum(x[1] for x in sbr[:b]) * 128
        rows.append((prev, r1))
        prev = r1
    return sbr, bounds, rows


def _pos_map(NPAD):
    """Chunk-major permuted node position: table[POS[v]] = x[v]."""
    NL = NPAD // NCORES
    B = NL // 128
    _, _, rows = _chunks(B)
    POS = np.empty(NPAD, np.int64)
    base = 0
    for (r0, r1) in rows:
        ch = r1 - r0
        for r in range(NCORES):
            v = r * NL + np.arange(r0, r1)
            POS[v] = base + r * ch + np.arange(ch)
        base += NCORES * ch
    return POS


def _prep_adjacency(src, dst, attr, NPAD, POS):
    NL = NPAD // NCORES
    B = NL // 128
    SR = NPAD // R
    per_core = []
    core = dst // NL
    pos = POS[src]
    for r in range(NCORES):
        m = core == r
        p = pos[m]
        d = (dst[m] - r * NL).astype(np.int64)
        a = attr[m].astype(np.float32)
        b = d >> 7
        drel = (d & 127).astype(np.float32)
        q = p // SR
        srel = (p - q * SR).astype(np.int16)
        key = (b * R + q).astype(np.int64)
        order = np.argsort(key, kind="stable")
        key_s = key[order]
        counts = np.bincount(key_s, minlength=B * R)
        starts = np.concatenate([[0], np.cumsum(counts)[:-1]])
        pos_in = np.arange(len(key_s)) - starts[key_s]
        per_core.append((key_s, pos_in, srel[order], drel[order], a[order], counts))
    max_count = max(int(pc[5].max()) for pc in per_core) if len(src) else 0
    return per_core, max_count


def _finalize_adjacency(per_core, CPR, NPAD):
    """idx: [128, IDXW] int16, columns ordered [sb][q][local block][slot][8].
    ohst: [128, B*CB*128] bf16 — host-built one-hot scatter grids; column
    layout [b][j][dest] with j = q*CPR + s, partition = slot-within-chunk."""
    NL = NPAD // NCORES
    B = NL // 128
    CB = R * CPR
    cap = CPR * 128
    sbr = _sb_ranges(B)
    idx_arrs, oh_arrs, gpos_arrs = [], [], []
    for key_s, pos_in, srel, drel, a, counts in per_core:
        grid_src = np.zeros((B, R, cap), np.int16)
        bq_b = key_s // R
        bq_q = key_s % R
        grid_src[bq_b, bq_q, pos_in] = srel
        s_ = pos_in // 128
        p_ = pos_in % 128
        d_ = drel.astype(np.int64)
        grid_oh = np.zeros((B, R, CPR, 128, 128), BF16)
        grid_oh[bq_b, bq_q, s_, p_, d_] = a.astype(BF16)
        segs = []
        gsegs = []
        SR = NPAD // R
        for b0, nb in sbr:
            blk = grid_src[b0:b0 + nb]            # [nb, R, cap]
            t = blk.transpose(1, 0, 2)            # [R, nb, cap]
            segs.append(t.reshape(-1))
            gq = np.repeat(np.arange(R, dtype=np.int64), nb * cap)
            gsegs.append(t.reshape(-1).astype(np.int64) + gq * SR)
        tokens = np.concatenate(segs)
        gpos_arrs.append(np.concatenate(gsegs))
        wrapped = np.tile(tokens.reshape(-1, 16).T, (8, 1))
        idx_arrs.append(np.ascontiguousarray(wrapped))
        oh = grid_oh.transpose(3, 0, 1, 2, 4).reshape(128, B * CB * 128)
        oh_arrs.append(np.ascontiguousarray(oh))
    return idx_arrs, oh_arrs, gpos_arrs


def _build_kernel(NPAD, CPR):
    NL = NPAD // NCORES
    B = NL // 128
    CB = R * CPR
    SR = NPAD // R
    sbr, cc_bounds, cc_rows = _chunks(B)
    IDXW = R * B * CPR * 8

    nc = bacc.Bacc("TRN2", target_bir_lowering=False, debug=False, num_devices=NCORES,
                   num_swdge_queues=4, dynamic_dma_scratch_size=SCRATCH)
    dt = mybir.dt
    x_table = nc.declare_dram_parameter("input0", [NPAD, F], dt.bfloat16, isOutput=False)
    xT0_in = nc.declare_dram_parameter("input1", [128, NL], dt.bfloat16, isOutput=False)
    idx_in = [
        nc.declare_dram_parameter(f"input{2 + i}", [128, IDXW], dt.int16, isOutput=False)
        for i in range(2)
    ]
    oh_in = [
        nc.declare_dram_parameter(f"input{4 + i}", [128, B * CB * 128],
                                  dt.bfloat16, isOutput=False)
        for i in range(2)
    ]
    w_in = nc.declare_dram_parameter("input6", [9 * 128, F], dt.bfloat16, isOutput=False)
    bias_in = nc.declare_dram_parameter("input7", [128, 3 * F], dt.float32, isOutput=False)
    ident_in = nc.declare_dram_parameter("input8", [128, 128], dt.bfloat16, isOutput=False)
    gstr_in = [
        nc.declare_dram_parameter(f"input{9 + i}", [128, R * B * CPR * F],
                                  dt.bfloat16, isOutput=False)
        for i in range(2)
    ]
    out_p = nc.declare_dram_parameter("output0", [NL, F], dt.float32, isOutput=True)

    table1 = nc.dram_tensor("table1", [NPAD, F], dt.bfloat16, addr_space="Shared")
    table2 = nc.dram_tensor("table2", [NPAD, F], dt.bfloat16, addr_space="Shared")
    shard = [nc.dram_tensor(f"shard{k}", [NL, F], dt.bfloat16) for k in range(2)]
    tables = [x_table, table1, table2]

    with tile.TileContext(nc) as tc:
        with (
            tc.tile_pool(name="persist", bufs=1) as pp,
            tc.tile_pool(name="idxp", bufs=2) as idxp,
            tc.tile_pool(name="g0", bufs=2) as gp0,
            tc.tile_pool(name="g1", bufs=2) as gp1,
            tc.tile_pool(name="g2", bufs=2) as gp2,
            tc.tile_pool(name="g3", bufs=2) as gp3,
            tc.tile_pool(name="ohp", bufs=4) as ohp,
            tc.tile_pool(name="stp", bufs=12) as stp,
            tc.tile_pool(name="outp", bufs=4) as outp,
            tc.tile_pool(name="psA", bufs=6, space="PSUM") as psA,
            tc.tile_pool(name="psB", bufs=2, space="PSUM") as psB,
        ):
            gpools = [gp0, gp1, gp2, gp3]
            ident_t = pp.tile([128, 128], dt.bfloat16, tag="ident")
            nc.sync.dma_start(ident_t[:], ident_in[:])
            w_t = pp.tile([128, 9, 128], dt.bfloat16, tag="w")
            nc.sync.dma_start(w_t[:], w_in[:].rearrange("(w i) o -> i w o", i=128))
            bias_t = pp.tile([128, 3 * F], dt.float32, tag="bias")
            nc.sync.dma_start(bias_t[:], bias_in[:])
            xT = pp.tile([128, NL], dt.bfloat16, tag="xT")
            nc.sync.dma_start(xT[:], xT0_in[:])

            for k in range(3):
                table = tables[k]
                sb_col0 = 0
                sb_slot0 = 0
                for sbi, (b0, nb) in enumerate(sbr):
                    ncols_q = nb * CPR * 8
                    gts = []
                    idx_ts = []
                    for a in range(2):
                        if k > 0:
                            idx_t = idxp.tile([128, R * ncols_q], dt.int16,
                                              tag=f"idx{a}", name=f"idx{a}")
                            nc.sync.dma_start(
                                idx_t[:], idx_in[a][:, sb_col0: sb_col0 + R * ncols_q])
                            idx_ts.append(idx_t)
                        gts.append([
                            gpools[q].tile([128, nb * CPR, F], dt.bfloat16,
                                           tag=f"g{a}{q}", name=f"g{a}{q}")
                            for q in range(R)
                        ])
                    nch_total = nb * CPR
                    if k == 0:
                        # Layer 1: mostly host-precomputed sequential streams;
                        # the L1_GATHER slices go through the otherwise-idle
                        # SWDGE gather path instead (same idx/table addressing
                        # as layers 2/3, source = the input node table).
                        gslices = [s for s in L1_GATHER]
                        if gslices:
                            idxg = idxp.tile([128, len(gslices) * ncols_q],
                                             dt.int16, tag="idxL1", name="idxL1")
                        for gi, (a, q) in enumerate(gslices):
                            nc.sync.dma_start(
                                idxg[:, gi * ncols_q:(gi + 1) * ncols_q],
                                idx_in[a][:, sb_col0 + q * ncols_q:
                                          sb_col0 + (q + 1) * ncols_q])
                        for a in range(2):
                            for q in range(R):
                                if (a, q) in L1_GATHER:
                                    continue
                                col = (sb_slot0 + q * nch_total) * F
                                nc.sync.dma_start(
                                    gts[a][q][:],
                                    gstr_in[a][:, col: col + nch_total * F])
                        for gi, (a, q) in enumerate(gslices):
                            c0 = 0
                            while c0 < nch_total:
                                ncall = min(8, nch_total - c0)
                                nc.gpsimd.dma_gather(
                                    out_ap=gts[a][q][:, c0: c0 + ncall, :],
                                    in_ap=x_table[q * SR: (q + 1) * SR, :],
                                    idxs_ap=idxg[:, gi * ncols_q + c0 * 8:
                                                 gi * ncols_q + (c0 + ncall) * 8],
                                    num_idxs=ncall * 128,
                                    num_idxs_reg=ncall * 128,
                                    elem_size=F,
                                    queue_num=q,
                                )
                                c0 += ncall
                    else:
                        c0 = 0
                        while c0 < nch_total:
                            ncall = min(8, nch_total - c0)
                            for a in range(2):
                                for q in range(R):
                                    nc.gpsimd.dma_gather(
                                        out_ap=gts[a][q][:, c0: c0 + ncall, :],
                                        in_ap=table[q * SR: (q + 1) * SR, :],
                                        idxs_ap=idx_ts[a][:, q * ncols_q + c0 * 8:
                                                          q * ncols_q + (c0 + ncall) * 8],
                                        num_idxs=ncall * 128,
                                        num_idxs_reg=ncall * 128,
                                        elem_size=F,
                                        queue_num=q,
                                    )
                            c0 += ncall
                    sb_col0 += R * ncols_q
                    sb_slot0 += R * nch_total
                    # compute: streamed one-hot + chunk matmuls per (a, block)
                    st_tiles = [[None] * nb for _ in range(2)]
                    for a in range(2):
                        for bl in range(nb):
                            b = b0 + bl
                            oh = ohp.tile([128, CB * 128], dt.bfloat16, tag="oh",
                                          name="oh")
                            nc.scalar.dma_start(
                                oh[:], oh_in[a][:, b * CB * 128: (b + 1) * CB * 128])
                            ps = psA.tile([128, 128], dt.float32, tag="psA", name="psA")
                            for j in range(CB):
                                q, s = divmod(j, CPR)
                                nc.tensor.matmul(
                                    ps[:], gts[a][q][:, bl * CPR + s, :],
                                    oh[:, j * 128: (j + 1) * 128],
                                    start=(j == 0), stop=(j == CB - 1),
                                )
                            st = stp.tile([128, 128], dt.bfloat16, tag=f"st{a}",
                                          name=f"st{a}")
                            nc.scalar.copy(st[:], ps[:])
                            st_tiles[a][bl] = st
                    # dense + bias + store per block
                    for bl in range(nb):
                        b = b0 + bl
                        sl = slice(b * 128, (b + 1) * 128)
                        po = psB.tile([128, F], dt.float32, tag="psB", name="psB")
                        nc.tensor.matmul(po[:], st_tiles[0][bl][:], w_t[:, k * 3 + 1, :],
                                         start=True, stop=False)
                        nc.tensor.matmul(po[:], st_tiles[1][bl][:], w_t[:, k * 3 + 2, :],
                                         start=False, stop=False)
                        nc.tensor.matmul(po[:], xT[:, sl], w_t[:, k * 3 + 0, :],
                                         start=False, stop=True)
                        if k < 2:
                            ob = outp.tile([128, F], dt.bfloat16, tag="ob_bf",
                                           name="ob_bf")
                            nc.vector.tensor_tensor(
                                out=ob[:], in0=po[:], in1=bias_t[:, k * F: (k + 1) * F],
                                op=mybir.AluOpType.add)
                            nc.sync.dma_start(shard[k][sl, :], ob[:])
                            pt = psB.tile([128, F], dt.float32, tag="psB", name="psT")
                            nc.tensor.matmul(pt[:], ob[:], ident_t[:],
                                             start=True, stop=True)
                            nc.scalar.copy(xT[:, sl], pt[:])
                        else:
                            ob = outp.tile([128, F], dt.float32, tag="ob_f32",
                                           name="ob_f32")
                            nc.vector.tensor_tensor(
                                out=ob[:], in0=po[:], in1=bias_t[:, k * F: (k + 1) * F],
                                op=mybir.AluOpType.add)
                            nc.sync.dma_start(out_p[sl, :], ob[:])
                    # chunked AllGather into the contiguous permuted table
                    if k < 2 and (sbi + 1) in cc_bounds:
                        ci = cc_bounds.index(sbi + 1)
                        r0, r1 = cc_rows[ci]
                        ch = r1 - r0
                        base = NCORES * r0
                        nc.gpsimd.collective_compute(
                            "AllGather",
                            mybir.AluOpType.bypass,
                            replica_groups=[list(range(NCORES))],
                            ins=[shard[k][r0:r1, :]],
                            outs=[tables[k + 1][base: base + NCORES * ch, :]],
                        )

    from concourse.tile_scheduler import PROC_NAME_TO_IDX
    idx2name = {v: k for k, v in PROC_NAME_TO_IDX.items()}
    for fn in nc.m.functions:
        for block in fn.blocks:
            for inst in block.instructions:
                if isinstance(inst, mybir.InstDMAGatherAnt):
                    pname = str(idx2name.get(inst.bass_scheduled_proc, ""))
                    if pname.startswith("DMASW"):
                        inst.queue_num = int(pname[5:]) % 4
    nc.finalize()
    return nc


def _run(x, edge_index, edge_attr, edge_index2, edge_attr2, weights, biases, NPAD,
         trace=False):
    NL = NPAD // NCORES
    n = x.shape[0]
    POS = _pos_map(NPAD)

    adjs = []
    maxc = 0
    for (src, dst), attr in ((edge_index, edge_attr), (edge_index2, edge_attr2)):
        pc, mc = _prep_adjacency(
            np.asarray(src, np.int64), np.asarray(dst, np.int64), attr, NPAD, POS)
        adjs.append(pc)
        maxc = max(maxc, mc)
    CPR = max(1, -(-maxc // 128))
    data = [_finalize_adjacency(pc, CPR, NPAD) for pc in adjs]
    NLB = NPAD // NCORES // 128

    xpad = np.zeros((NPAD, x.shape[1]), np.float32)
    xpad[:n] = x
    xtab = np.zeros((NPAD, x.shape[1]), np.float32)
    xtab[POS] = xpad
    xtab = xtab.astype(BF16)
    xpad_bf = xpad.astype(BF16)

    wstack = np.concatenate(
        [np.asarray(w, np.float32) for trio in weights for w in trio], axis=0
    ).astype(BF16)
    bstack = np.concatenate(
        [np.tile(np.asarray(b, np.float32)[None, :], (128, 1)) for b in biases], axis=1
    ).astype(np.float32)
    ident = np.eye(128, dtype=np.float32).astype(BF16)

    S = R * NLB * CPR  # slots per adjacency
    in_maps = []
    for r in range(NCORES):
        xT0 = np.ascontiguousarray(xpad_bf[r * NL: (r + 1) * NL].T)
        gstr = []
        for a in range(2):
            gpos = data[a][2][r]
            rows = xtab[gpos]                       # [S*128, F] bf16
            gstr.append(np.ascontiguousarray(
                rows.reshape(S, 128, F).transpose(1, 0, 2).reshape(128, S * F)))
        in_maps.append(
            {
                "input0": xtab,
                "input1": xT0,
                "input2": data[0][0][r],
                "input3": data[1][0][r],
                "input4": data[0][1][r],
                "input5": data[1][1][r],
                "input6": wstack,
                "input7": bstack,
                "input8": ident,
                "input9": gstr[0],
                "input10": gstr[1],
            }
        )

    nc = _build_kernel(NPAD, CPR)
    res = run_bass_kernel_spmd(nc, in_maps, list(range(NCORES)), trace=trace)
    out = np.concatenate([res.results[r]["output0"] for r in range(NCORES)], axis=0)
    return out[:n], res


def kernel(**inputs):
    x = np.asarray(inputs["x"], np.float32)
    weights = []
    biases = []
    for blk in ("b1", "b2", "b3"):
        weights.append(
            (
                np.asarray(inputs[f"{blk}_ln_w"], np.float32),
                np.asarray(inputs[f"{blk}_c1_w"], np.float32),
                np.asarray(inputs[f"{blk}_c2_w"], np.float32),
            )
        )
        biases.append(
            np.asarray(inputs[f"{blk}_ln_b"], np.float32)
            + np.asarray(inputs[f"{blk}_c1_b"], np.float32)
            + np.asarray(inputs[f"{blk}_c2_b"], np.float32)
        )
    out, _ = _run(
        x,
        np.asarray(inputs["edge_index"]),
        np.asarray(inputs["edge_attr"], np.float32),
        np.asarray(inputs["edge_index2"]),
        np.asarray(inputs["edge_attr2"], np.float32),
        weights,
        biases,
        NPAD,
    )
    return out
